# revision 11
# baseline (speedup 1.0000x reference)
"""MoE top-1 routing kernel for Trainium2 (8 NeuronCores, expert-F-sharded).

Model (E=8, D=512, F=2048, N=4096):
    logits = x @ Wg + bg; e = argmax(logits)
    y[i] = relu(x[i] @ W1[e] + b1[e]) @ W2[e] + b2[e]

Strategy (v6 — quarter-F expert pairing + queue re-routing):
- Host computes the gate (f64 matmul + argmax) and routes tokens.
- Each expert's FFN is split into 4 F-quarters (Fs=512). Experts are paired
  (adjacent in sorted-count order) into 4 "slots"; slot s appears on every
  core with the same compile-time token width W_s = max count over the
  slot's two experts. Core j, slot s holds (expert = pair[s][j//4],
  quarter q = j%4) and processes ALL of that expert's tokens against its
  F-quarter. PE work per core = 32*sum_s W_s cycles; adjacent-pairing
  minimizes sum of pair maxima, so expert imbalance costs only ~4%.
- Everything on the wire is bf16; PSUM accumulates fp32; b1 is applied in
  the Relu, b2 is added only by the q==0 cell (zeros elsewhere), partial
  y's are summed on the host in fp32.
- Queue routing (the v5 lesson): the scalar/vector queues must not sit
  behind bulk DMA issue, or the PSUM-pool rotation stalls the matmul
  stream and HAM down-clocks the PE. w pieces go on sync; chunk-0 x ko0
  and b on gpsimd (SWDGE, small); chunk-0 x rest upfront on scalar; later
  x pieces on scalar emitted one chunk ahead of consumption. PSUM->SBUF
  element ops split by parity between scalar ACTIVATE and vector dual-op
  TENSOR_SCALAR.
- DMA completions have a ~3-5us head latency (single hardware read queue,
  out-of-order packet completion): 26 warm-up matmuls keep the PE busy
  (and the HAM clock ramping 1.2->2.4GHz) until chunk-0 data + semaphore
  reliably land.
- y is chunk-major so every output DMA is contiguous per partition;
  non-last chunks ship as one DMA on sync; the smallest chunk runs last
  and drains per-d across engines.
"""

import sys

sys.path.insert(0, "/opt/trn_rl_repo")

import numpy as np
import ml_dtypes

BF16 = ml_dtypes.bfloat16
E, D, F, N_CORES = 8, 512, 2048, 8
KD = D // 128      # 4 contraction tiles (stage1) == output d tiles (stage2)
FS = 512           # F-columns per slot (quarter of F)
KQ = FS // 128     # 4
NSLOT = 4
N_WARM = 16

_cache: dict = {}


def _chunks_of(w: int) -> list[int]:
    # split width into <=512-col chunks (PSUM bank limit), evenly
    if w <= 512:
        return [w]
    n = -(-w // 512)
    base = (-(-w // n) + 15) // 16 * 16
    out, rem = [], w
    while rem > 0:
        c = min(base, rem)
        out.append(c)
        rem -= c
    return out


def _chunk_list(widths):
    """Chunk schedule: (slot, lo, cw, off) in execution order; the
    smallest chunk is moved to the end to shorten the drain tail."""
    ch = []
    for s, w in enumerate(widths):
        lo = 0
        for cw in _chunks_of(w):
            ch.append([s, lo, cw])
            lo += cw
    k = min(range(len(ch)), key=lambda i: (ch[i][2], -i))
    ch.append(ch.pop(k))
    off = 0
    out = []
    for s, lo, cw in ch:
        out.append((s, lo, cw, off))  # x and y share chunk-major offsets
        off += KD * cw
    return out, off


def _build(widths: tuple[int, ...]):
    import concourse.tile as tile
    import concourse.mybir as mybir
    from concourse import bacc

    f32 = mybir.dt.float32
    bf16 = mybir.dt.bfloat16
    Relu = mybir.ActivationFunctionType.Relu
    Ident = mybir.ActivationFunctionType.Identity
    Add = mybir.AluOpType.add
    Max = mybir.AluOpType.max

    nc = bacc.Bacc("TRN2", target_bir_lowering=False, debug=False)

    CH, xtot = _chunk_list(widths)
    n_ch = len(CH)

    # Layouts (all [128, *]):
    #   w[p, s*4096 + f*512 + ko*128 + c]        = W1[e][128*ko+p, 512*q + 128*f + c]
    #   w[p, s*4096 + 2048 + d*512 + fo*128 + c] = W2[e][512*q + 128*fo + p, 128*d + c]
    #   x[p, xoff + ko*cw + c]                   = x[tok_{lo+c}, 128*ko + p]
    #   b[p, s*8+f] = b1[e][512*q+128*f+p];  b[p, s*8+4+d] = b2[e][128*d+p] (q==0 else 0)
    #   y[p, yoff + d*cw + c]                    = partial y[tok_{lo+c}, 128*d+p]
    w_d = nc.dram_tensor("w", [128, NSLOT * 4096], bf16, kind="ExternalInput").ap()
    x_d = nc.dram_tensor("x", [128, xtot], bf16, kind="ExternalInput").ap()
    b_d = nc.dram_tensor("b", [128, NSLOT * 8], f32, kind="ExternalInput").ap()
    y_d = nc.dram_tensor("y", [128, xtot], bf16, kind="ExternalOutput").ap()

    # Emission plan: st1(i+1) between st1(i) and st2(i)
    plan = [("st1", 0)]
    for i in range(n_ch):
        if i + 1 < n_ch:
            plan.append(("st1", i + 1))
        plan.append(("st2", i))

    # Input pieces, all on the sync queue in consumption order. The HWDGE
    # completion-semaphore pool holds only ~8 DMAs; beyond that the compiler
    # recycles sems and upgrades consumer waits to LATER pieces on the FIFO
    # ring (false dependencies). So: few, large pieces (slot-granular w,
    # chunk-group-granular x), ordered by first-consumption position so the
    # single ~350GB/s read stream arrives just ahead of the matmuls.
    # pieces = ("w"|"x"|"b", lo, hi).
    pos1 = {ci: p for p, (op, ci) in enumerate(plan) if op == "st1"}
    pos2 = {ci: p for p, (op, ci) in enumerate(plan) if op == "st2"}
    s0 = CH[0][0]
    pieces = [("w", s0 * 4096, s0 * 4096 + 512),
              ("b", 0, NSLOT * 8),
              ("x", CH[0][3], CH[0][3] + KD * CH[0][2]),
              ("w", s0 * 4096 + 512, s0 * 4096 + 2048)]
    events = [(pos2[0], 1, ("w", s0 * 4096 + 2048, s0 * 4096 + 4096))]
    wseen = {s0}
    ci = 1
    while ci < n_ch:
        grp = [ci]
        # extend the x-group up to (and including) the next new-slot chunk
        while CH[grp[-1]][0] in wseen and grp[-1] + 1 < n_ch:
            grp.append(grp[-1] + 1)
        new_s = CH[grp[-1]][0]
        new_s = None if new_s in wseen else new_s
        # absorb trailing chunks whose weights are already covered
        while grp[-1] + 1 < n_ch and \
                CH[grp[-1] + 1][0] in (wseen | {new_s}):
            grp.append(grp[-1] + 1)
        last = CH[grp[-1]]
        events.append((pos1[grp[0]], 0,
                       ("x", CH[grp[0]][3], last[3] + KD * last[2])))
        if new_s is not None:
            wseen.add(new_s)
            events.append((pos1[[c for c in grp if CH[c][0] == new_s][0]], 1,
                           ("w", new_s * 4096, new_s * 4096 + 4096)))
        ci = grp[-1] + 1
    events.sort()
    pieces += [ev for _, _, ev in events]

    with tile.TileContext(nc) as tc:
        with tc.tile_pool(name="wp", bufs=1) as wp, \
             tc.tile_pool(name="hp", bufs=2) as hp, \
             tc.tile_pool(name="yp", bufs=2) as yp, \
             tc.tile_pool(name="scr", bufs=1) as scr, \
             tc.tile_pool(name="pp", bufs=3, space="PSUM") as pp:

            # --- PE warm-up: dummy matmuls bridge the DMA head latency and
            # keep the HAM clock ramp going until chunk-0 data lands.
            wrm = scr.tile([128, 256], bf16, name="wrm")
            nc.gpsimd.memset(wrm[:], 0.0)
            wps = pp.tile([128, 256], f32, name="wps", tag="wps", bufs=1)
            for _ in range(N_WARM):
                nc.tensor.matmul(wps[:], wrm[:, :128], wrm[:], start=True, stop=True)

            wt = wp.tile([128, NSLOT * 4096], bf16, name="wt")
            xt = wp.tile([128, xtot], bf16, name="xt")
            bis = wp.tile([128, NSLOT * 8], f32, name="bis")

            # --- head DMA issue: everything on sync, consumption order ---
            for kind, lo, hi in pieces:
                t, dr = {"w": (wt, w_d), "x": (xt, x_d), "b": (bis, b_d)}[kind]
                nc.sync.dma_start(t[:, lo:hi], dr[:, lo:hi])

            # --- compute ---
            hs = {}

            def st1(ci):
                s, lo, cw, off = CH[ci]
                for f in range(KQ):
                    p1 = pp.tile([128, 512], f32, name=f"p1_{ci}_{f}", tag="p1")
                    for ko in range(KD):
                        lhsT = wt[:, s * 4096 + f * 512 + ko * 128:
                                  s * 4096 + f * 512 + ko * 128 + 128]
                        rhs = xt[:, off + ko * cw: off + (ko + 1) * cw]
                        nc.tensor.matmul(p1[:, :cw], lhsT, rhs,
                                         start=(ko == 0), stop=(ko == KD - 1))
                    h = hp.tile([128, 512], bf16, name=f"h{ci}_{f}", tag=f"h{f}")
                    bcol = bis[:, s * 8 + f: s * 8 + f + 1]
                    if f % 2 == 0:
                        nc.scalar.activation(h[:, :cw], p1[:, :cw], Relu, bias=bcol)
                    else:
                        nc.vector.tensor_scalar(h[:, :cw], p1[:, :cw], bcol, 0.0,
                                                Add, Max)
                    hs[(ci, f)] = h

            def st2(ci, last):
                s, lo, cw, off = CH[ci]
                ys = yp.tile([128, KD * 512], bf16, name=f"ys{ci}", tag="ys")
                for d in range(KD):
                    p2 = pp.tile([128, 512], f32, name=f"p2_{ci}_{d}",
                                 tag=f"p2_{d}", bufs=1)
                    for fo in range(KQ):
                        lhsT = wt[:, s * 4096 + 2048 + d * 512 + fo * 128:
                                  s * 4096 + 2048 + d * 512 + fo * 128 + 128]
                        nc.tensor.matmul(p2[:, :cw], lhsT, hs[(ci, fo)][:, :cw],
                                         start=(fo == 0), stop=(fo == KQ - 1))
                    bcol = bis[:, s * 8 + 4 + d: s * 8 + 4 + d + 1]
                    if d % 2 == 0:
                        nc.scalar.activation(ys[:, d * cw:(d + 1) * cw],
                                             p2[:, :cw], Ident, bias=bcol)
                    else:
                        nc.vector.tensor_scalar_add(ys[:, d * cw:(d + 1) * cw],
                                                    p2[:, :cw], bcol)
                    if last:
                        eng = [nc.sync, nc.scalar, nc.sync, nc.scalar][d]
                        eng.dma_start(y_d[:, off + d * cw: off + (d + 1) * cw],
                                      ys[:, d * cw:(d + 1) * cw])
                if not last:
                    nc.sync.dma_start(y_d[:, off: off + KD * cw],
                                      ys[:, :KD * cw])

            for op, ci in plan:
                if op == "st1":
                    st1(ci)
                else:
                    st2(ci, last=(ci == n_ch - 1))

    nc.compile()
    return nc


def _get_nc(widths: tuple[int, ...]):
    if widths not in _cache:
        _cache[widths] = _build(widths)
    return _cache[widths]


def _plan(counts):
    """Pair adjacent experts in sorted order into NSLOT slots (minimizes
    sum of per-slot maxima); return (pairs, widths)."""
    order = np.argsort(-counts, kind="stable")
    pairs = [(int(order[2 * s]), int(order[2 * s + 1])) for s in range(NSLOT)]
    widths = tuple(
        (max(int(counts[a]), int(counts[b]), 16) + 15) // 16 * 16
        for a, b in pairs)
    return pairs, widths


def _pack_inputs(x, W1, b1, W2, b2, order, starts, pairs, widths):
    """Build per-core in_maps. Core j, slot s: expert pair[s][j//4], quarter j%4."""
    CH, xtot = _chunk_list(widths)
    xbf = x.astype(BF16)
    toks_of = [order[starts[e]:starts[e + 1]] for e in range(E)]
    in_maps = []
    for j in range(N_CORES):
        q = j % 4
        wcols = np.empty((128, NSLOT * 4096), BF16)
        bcols = np.zeros((128, NSLOT * 8), np.float32)
        xcols = np.zeros((128, xtot), BF16)
        xe_cache = {}
        for s in range(NSLOT):
            e = pairs[s][0] if j < 4 else pairs[s][1]
            # w1 (f-major): [p, f*512 + ko*128 + c]
            w1s = W1[e][:, FS * q: FS * (q + 1)]               # [D, Fs]
            wcols[:, s * 4096: s * 4096 + 2048] = \
                w1s.reshape(KD, 128, KQ, 128).transpose(1, 2, 0, 3).reshape(128, KD * FS)
            # w2 (d-major): [p, d*512 + fo*128 + c]
            w2s = W2[e][FS * q: FS * (q + 1), :]               # [Fs, D]
            wcols[:, s * 4096 + 2048: s * 4096 + 4096] = \
                w2s.reshape(KQ, 128, KD, 128).transpose(1, 2, 0, 3).reshape(128, KQ * D)
            bcols[:, s * 8: s * 8 + KQ] = b1[e][FS * q: FS * (q + 1)].reshape(KQ, 128).T
            if q == 0:
                bcols[:, s * 8 + 4: s * 8 + 8] = b2[e].reshape(KD, 128).T
            toks = toks_of[e]
            xe = np.zeros((widths[s], D), BF16)
            xe[:len(toks)] = xbf[toks]
            xe_cache[s] = xe.T                                  # [D, W]
        for s, lo, cw, off in CH:
            xcols[:, off: off + KD * cw] = \
                xe_cache[s][:, lo:lo + cw].reshape(KD, 128, cw) \
                .transpose(1, 0, 2).reshape(128, KD * cw)
        in_maps.append({
            "w": np.ascontiguousarray(wcols),
            "x": np.ascontiguousarray(xcols),
            "b": bcols,
        })
    return in_maps, toks_of


def kernel(x, Wg, bg, W1, b1, W2, b2):
    from concourse.bass_utils import run_bass_kernel_spmd

    x = np.asarray(x, dtype=np.float32)
    n_tok = x.shape[0]

    # host gate in f64: the mathematically-true argmax
    logits = x.astype(np.float64) @ np.asarray(Wg, np.float64) + np.asarray(bg, np.float64)
    idx = logits.argmax(1)

    counts = np.bincount(idx, minlength=E)
    order = np.argsort(idx, kind="stable")
    starts = np.zeros(E + 1, np.int64)
    starts[1:] = np.cumsum(counts)

    pairs, widths = _plan(counts)

    W1 = np.asarray(W1, np.float32)
    W2 = np.asarray(W2, np.float32)
    b1 = np.asarray(b1, np.float32)
    b2 = np.asarray(b2, np.float32)

    in_maps, toks_of = _pack_inputs(x, W1, b1, W2, b2, order, starts, pairs, widths)
    nc = _get_nc(widths)
    res = run_bass_kernel_spmd(nc, in_maps, core_ids=list(range(N_CORES)))

    CH, xtot = _chunk_list(widths)
    out = np.zeros((n_tok, D), np.float32)
    for j in range(N_CORES):
        yv = res.results[j]["y"]
        for s, lo, cw, off in CH:
            e = pairs[s][0] if j < 4 else pairs[s][1]
            toks = toks_of[e]
            seg = toks[lo:lo + cw]
            if len(seg) == 0:
                continue
            blk = yv[:, off: off + KD * cw].astype(np.float32) \
                .reshape(128, KD, cw).transpose(2, 1, 0).reshape(cw, D)
            out[seg] += blk[:len(seg)]
    return out


# revision 12
# speedup vs baseline: 1.0361x; 1.0361x over previous
"""MoE top-1 routing kernel for Trainium2 (8 NeuronCores, expert-F-sharded).

Model (E=8, D=512, F=2048, N=4096):
    logits = x @ Wg + bg; e = argmax(logits)
    y[i] = relu(x[i] @ W1[e] + b1[e]) @ W2[e] + b2[e]

Strategy (v6 — quarter-F expert pairing + queue re-routing):
- Host computes the gate (f64 matmul + argmax) and routes tokens.
- Each expert's FFN is split into 4 F-quarters (Fs=512). Experts are paired
  (adjacent in sorted-count order) into 4 "slots"; slot s appears on every
  core with the same compile-time token width W_s = max count over the
  slot's two experts. Core j, slot s holds (expert = pair[s][j//4],
  quarter q = j%4) and processes ALL of that expert's tokens against its
  F-quarter. PE work per core = 32*sum_s W_s cycles; adjacent-pairing
  minimizes sum of pair maxima, so expert imbalance costs only ~4%.
- Everything on the wire is bf16; PSUM accumulates fp32; b1 is applied in
  the Relu, b2 is added only by the q==0 cell (zeros elsewhere), partial
  y's are summed on the host in fp32.
- Queue routing (the v5 lesson): the scalar/vector queues must not sit
  behind bulk DMA issue, or the PSUM-pool rotation stalls the matmul
  stream and HAM down-clocks the PE. w pieces go on sync; chunk-0 x ko0
  and b on gpsimd (SWDGE, small); chunk-0 x rest upfront on scalar; later
  x pieces on scalar emitted one chunk ahead of consumption. PSUM->SBUF
  element ops split by parity between scalar ACTIVATE and vector dual-op
  TENSOR_SCALAR.
- DMA completions have a ~3-5us head latency (single hardware read queue,
  out-of-order packet completion): 26 warm-up matmuls keep the PE busy
  (and the HAM clock ramping 1.2->2.4GHz) until chunk-0 data + semaphore
  reliably land.
- y is chunk-major so every output DMA is contiguous per partition;
  non-last chunks ship as one DMA on sync; the smallest chunk runs last
  and drains per-d across engines.
"""

import sys

sys.path.insert(0, "/opt/trn_rl_repo")

import numpy as np
import ml_dtypes

BF16 = ml_dtypes.bfloat16
E, D, F, N_CORES = 8, 512, 2048, 8
KD = D // 128      # 4 contraction tiles (stage1) == output d tiles (stage2)
FS = 512           # F-columns per slot (quarter of F)
KQ = FS // 128     # 4
NSLOT = 4
N_WARM = 22

_cache: dict = {}


def _chunks_of(w: int) -> list[int]:
    # split width into <=512-col chunks (PSUM bank limit), evenly
    if w <= 512:
        return [w]
    n = -(-w // 512)
    base = (-(-w // n) + 15) // 16 * 16
    out, rem = [], w
    while rem > 0:
        c = min(base, rem)
        out.append(c)
        rem -= c
    return out


def _chunk_list(widths):
    """Chunk schedule: (slot, lo, cw, off) in execution order; the
    smallest chunk is moved to the end to shorten the drain tail."""
    ch = []
    for s, w in enumerate(widths):
        lo = 0
        for cw in _chunks_of(w):
            ch.append([s, lo, cw])
            lo += cw
    k = min(range(len(ch)), key=lambda i: (ch[i][2], -i))
    ch.append(ch.pop(k))
    off = 0
    out = []
    for s, lo, cw in ch:
        out.append((s, lo, cw, off))  # x and y share chunk-major offsets
        off += KD * cw
    return out, off


def _build(widths: tuple[int, ...]):
    import concourse.tile as tile
    import concourse.mybir as mybir
    from concourse import bacc

    f32 = mybir.dt.float32
    bf16 = mybir.dt.bfloat16
    Relu = mybir.ActivationFunctionType.Relu
    Ident = mybir.ActivationFunctionType.Identity
    Add = mybir.AluOpType.add
    Max = mybir.AluOpType.max

    nc = bacc.Bacc("TRN2", target_bir_lowering=False, debug=False)

    CH, xtot = _chunk_list(widths)
    n_ch = len(CH)

    # Layouts (all [128, *]):
    #   w[p, s*4096 + f*512 + ko*128 + c]        = W1[e][128*ko+p, 512*q + 128*f + c]
    #   w[p, s*4096 + 2048 + d*512 + fo*128 + c] = W2[e][512*q + 128*fo + p, 128*d + c]
    #   x[p, xoff + ko*cw + c]                   = x[tok_{lo+c}, 128*ko + p]
    #   b[p, s*8+f] = b1[e][512*q+128*f+p];  b[p, s*8+4+d] = b2[e][128*d+p] (q==0 else 0)
    #   y[p, yoff + d*cw + c]                    = partial y[tok_{lo+c}, 128*d+p]
    w_d = nc.dram_tensor("w", [128, NSLOT * 4096], bf16, kind="ExternalInput").ap()
    x_d = nc.dram_tensor("x", [128, xtot], bf16, kind="ExternalInput").ap()
    b_d = nc.dram_tensor("b", [128, NSLOT * 8], f32, kind="ExternalInput").ap()
    y_d = nc.dram_tensor("y", [128, xtot], bf16, kind="ExternalOutput").ap()

    # Emission plan: st1(i+1) between st1(i) and st2(i)
    plan = [("st1", 0)]
    for i in range(n_ch):
        if i + 1 < n_ch:
            plan.append(("st1", i + 1))
        plan.append(("st2", i))

    # Input pieces, all on the sync queue in consumption order. The HWDGE
    # completion-semaphore pool holds only ~8 DMAs; beyond that the compiler
    # recycles sems and upgrades consumer waits to LATER pieces on the FIFO
    # ring (false dependencies). So: few, large pieces (slot-granular w,
    # chunk-group-granular x), ordered by first-consumption position so the
    # single ~350GB/s read stream arrives just ahead of the matmuls.
    # pieces = ("w"|"x"|"b", lo, hi).
    pos1 = {ci: p for p, (op, ci) in enumerate(plan) if op == "st1"}
    pos2 = {ci: p for p, (op, ci) in enumerate(plan) if op == "st2"}
    s0 = CH[0][0]
    pieces = [("w", s0 * 4096, s0 * 4096 + 512),
              ("b", 0, NSLOT * 8),
              ("x", CH[0][3], CH[0][3] + KD * CH[0][2]),
              ("w", s0 * 4096 + 512, s0 * 4096 + 2048)]
    events = [(pos2[0], 1, ("w", s0 * 4096 + 2048, s0 * 4096 + 4096))]
    wseen = {s0}
    ci = 1
    while ci < n_ch:
        grp = [ci]
        # extend the x-group up to (and including) the next new-slot chunk
        while CH[grp[-1]][0] in wseen and grp[-1] + 1 < n_ch:
            grp.append(grp[-1] + 1)
        new_s = CH[grp[-1]][0]
        new_s = None if new_s in wseen else new_s
        # absorb trailing chunks whose weights are already covered
        while grp[-1] + 1 < n_ch and \
                CH[grp[-1] + 1][0] in (wseen | {new_s}):
            grp.append(grp[-1] + 1)
        last = CH[grp[-1]]
        events.append((pos1[grp[0]], 0,
                       ("x", CH[grp[0]][3], last[3] + KD * last[2])))
        if new_s is not None:
            wseen.add(new_s)
            events.append((pos1[[c for c in grp if CH[c][0] == new_s][0]], 1,
                           ("w", new_s * 4096, new_s * 4096 + 4096)))
        ci = grp[-1] + 1
    events.sort()
    pieces += [ev for _, _, ev in events]

    with tile.TileContext(nc) as tc:
        with tc.tile_pool(name="wp", bufs=1) as wp, \
             tc.tile_pool(name="hp", bufs=2) as hp, \
             tc.tile_pool(name="yp", bufs=2) as yp, \
             tc.tile_pool(name="scr", bufs=1) as scr, \
             tc.tile_pool(name="pp", bufs=3, space="PSUM") as pp:

            # --- PE warm-up: dummy matmuls bridge the DMA head latency and
            # keep the HAM clock ramp going until chunk-0 data lands.
            wrm = scr.tile([128, 256], bf16, name="wrm")
            nc.gpsimd.memset(wrm[:], 0.0)
            wps = pp.tile([128, 256], f32, name="wps", tag="wps", bufs=1)
            for _ in range(N_WARM):
                nc.tensor.matmul(wps[:], wrm[:, :128], wrm[:], start=True, stop=True)

            wt = wp.tile([128, NSLOT * 4096], bf16, name="wt")
            xt = wp.tile([128, xtot], bf16, name="xt")
            bis = wp.tile([128, NSLOT * 8], f32, name="bis")

            # --- head DMA issue: everything on sync, consumption order ---
            for kind, lo, hi in pieces:
                t, dr = {"w": (wt, w_d), "x": (xt, x_d), "b": (bis, b_d)}[kind]
                nc.sync.dma_start(t[:, lo:hi], dr[:, lo:hi])

            # --- compute ---
            hs = {}

            def st1(ci):
                s, lo, cw, off = CH[ci]
                for f in range(KQ):
                    p1 = pp.tile([128, 512], f32, name=f"p1_{ci}_{f}", tag="p1")
                    for ko in range(KD):
                        lhsT = wt[:, s * 4096 + f * 512 + ko * 128:
                                  s * 4096 + f * 512 + ko * 128 + 128]
                        rhs = xt[:, off + ko * cw: off + (ko + 1) * cw]
                        nc.tensor.matmul(p1[:, :cw], lhsT, rhs,
                                         start=(ko == 0), stop=(ko == KD - 1))
                    h = hp.tile([128, 512], bf16, name=f"h{ci}_{f}", tag=f"h{f}")
                    bcol = bis[:, s * 8 + f: s * 8 + f + 1]
                    if f % 2 == 0:
                        nc.scalar.activation(h[:, :cw], p1[:, :cw], Relu, bias=bcol)
                    else:
                        nc.vector.tensor_scalar(h[:, :cw], p1[:, :cw], bcol, 0.0,
                                                Add, Max)
                    hs[(ci, f)] = h

            def st2(ci, last):
                s, lo, cw, off = CH[ci]
                ys = yp.tile([128, KD * 512], bf16, name=f"ys{ci}", tag="ys")
                for d in range(KD):
                    p2 = pp.tile([128, 512], f32, name=f"p2_{ci}_{d}",
                                 tag=f"p2_{d}", bufs=1)
                    for fo in range(KQ):
                        lhsT = wt[:, s * 4096 + 2048 + d * 512 + fo * 128:
                                  s * 4096 + 2048 + d * 512 + fo * 128 + 128]
                        nc.tensor.matmul(p2[:, :cw], lhsT, hs[(ci, fo)][:, :cw],
                                         start=(fo == 0), stop=(fo == KQ - 1))
                    bcol = bis[:, s * 8 + 4 + d: s * 8 + 4 + d + 1]
                    if d % 2 == 0:
                        nc.scalar.activation(ys[:, d * cw:(d + 1) * cw],
                                             p2[:, :cw], Ident, bias=bcol)
                    else:
                        nc.vector.tensor_scalar_add(ys[:, d * cw:(d + 1) * cw],
                                                    p2[:, :cw], bcol)
                    if last:
                        eng = [nc.sync, nc.scalar, nc.sync, nc.scalar][d]
                        eng.dma_start(y_d[:, off + d * cw: off + (d + 1) * cw],
                                      ys[:, d * cw:(d + 1) * cw])
                if not last:
                    nc.sync.dma_start(y_d[:, off: off + KD * cw],
                                      ys[:, :KD * cw])

            for op, ci in plan:
                if op == "st1":
                    st1(ci)
                else:
                    st2(ci, last=(ci == n_ch - 1))

    nc.compile()
    return nc


def _get_nc(widths: tuple[int, ...]):
    if widths not in _cache:
        _cache[widths] = _build(widths)
    return _cache[widths]


def _plan(counts):
    """Pair adjacent experts in sorted order into NSLOT slots (minimizes
    sum of per-slot maxima); return (pairs, widths)."""
    order = np.argsort(-counts, kind="stable")
    pairs = [(int(order[2 * s]), int(order[2 * s + 1])) for s in range(NSLOT)]
    widths = tuple(
        (max(int(counts[a]), int(counts[b]), 16) + 15) // 16 * 16
        for a, b in pairs)
    return pairs, widths


def _pack_inputs(x, W1, b1, W2, b2, order, starts, pairs, widths):
    """Build per-core in_maps. Core j, slot s: expert pair[s][j//4], quarter j%4."""
    CH, xtot = _chunk_list(widths)
    xbf = x.astype(BF16)
    toks_of = [order[starts[e]:starts[e + 1]] for e in range(E)]
    in_maps = []
    for j in range(N_CORES):
        q = j % 4
        wcols = np.empty((128, NSLOT * 4096), BF16)
        bcols = np.zeros((128, NSLOT * 8), np.float32)
        xcols = np.zeros((128, xtot), BF16)
        xe_cache = {}
        for s in range(NSLOT):
            e = pairs[s][0] if j < 4 else pairs[s][1]
            # w1 (f-major): [p, f*512 + ko*128 + c]
            w1s = W1[e][:, FS * q: FS * (q + 1)]               # [D, Fs]
            wcols[:, s * 4096: s * 4096 + 2048] = \
                w1s.reshape(KD, 128, KQ, 128).transpose(1, 2, 0, 3).reshape(128, KD * FS)
            # w2 (d-major): [p, d*512 + fo*128 + c]
            w2s = W2[e][FS * q: FS * (q + 1), :]               # [Fs, D]
            wcols[:, s * 4096 + 2048: s * 4096 + 4096] = \
                w2s.reshape(KQ, 128, KD, 128).transpose(1, 2, 0, 3).reshape(128, KQ * D)
            bcols[:, s * 8: s * 8 + KQ] = b1[e][FS * q: FS * (q + 1)].reshape(KQ, 128).T
            if q == 0:
                bcols[:, s * 8 + 4: s * 8 + 8] = b2[e].reshape(KD, 128).T
            toks = toks_of[e]
            xe = np.zeros((widths[s], D), BF16)
            xe[:len(toks)] = xbf[toks]
            xe_cache[s] = xe.T                                  # [D, W]
        for s, lo, cw, off in CH:
            xcols[:, off: off + KD * cw] = \
                xe_cache[s][:, lo:lo + cw].reshape(KD, 128, cw) \
                .transpose(1, 0, 2).reshape(128, KD * cw)
        in_maps.append({
            "w": np.ascontiguousarray(wcols),
            "x": np.ascontiguousarray(xcols),
            "b": bcols,
        })
    return in_maps, toks_of


def kernel(x, Wg, bg, W1, b1, W2, b2):
    from concourse.bass_utils import run_bass_kernel_spmd

    x = np.asarray(x, dtype=np.float32)
    n_tok = x.shape[0]

    # host gate in f64: the mathematically-true argmax
    logits = x.astype(np.float64) @ np.asarray(Wg, np.float64) + np.asarray(bg, np.float64)
    idx = logits.argmax(1)

    counts = np.bincount(idx, minlength=E)
    order = np.argsort(idx, kind="stable")
    starts = np.zeros(E + 1, np.int64)
    starts[1:] = np.cumsum(counts)

    pairs, widths = _plan(counts)

    W1 = np.asarray(W1, np.float32)
    W2 = np.asarray(W2, np.float32)
    b1 = np.asarray(b1, np.float32)
    b2 = np.asarray(b2, np.float32)

    in_maps, toks_of = _pack_inputs(x, W1, b1, W2, b2, order, starts, pairs, widths)
    nc = _get_nc(widths)
    res = run_bass_kernel_spmd(nc, in_maps, core_ids=list(range(N_CORES)))

    CH, xtot = _chunk_list(widths)
    out = np.zeros((n_tok, D), np.float32)
    for j in range(N_CORES):
        yv = res.results[j]["y"]
        for s, lo, cw, off in CH:
            e = pairs[s][0] if j < 4 else pairs[s][1]
            toks = toks_of[e]
            seg = toks[lo:lo + cw]
            if len(seg) == 0:
                continue
            blk = yv[:, off: off + KD * cw].astype(np.float32) \
                .reshape(128, KD, cw).transpose(2, 1, 0).reshape(cw, D)
            out[seg] += blk[:len(seg)]
    return out
